# revision 6
# baseline (speedup 1.0000x reference)
"""Causal self-attention on 8 Trainium2 NeuronCores.

Problem: x[4,2048,1024] fp32, w_qkv[1024,3072], b_qkv[3072], w_out[1024,1024],
b_out[1024]; 16 heads, d_head 64; out = softmax_causal(QK^T/8) V @ w_out + b_out.

Sharding (hardcoded): core c handles batch b=c//2 and head-group g=c%2
(8 of 16 heads). Each core runs the full pipeline for its (batch, head-shard):
QKV projection, causal attention, and a partial output projection over its
512 head-channels. The host sums the two partial out-projections per batch
and adds b_out.

On-chip layout is "transposed": activations live as [channels, tokens] so
every matmul contracts over the partition dim.

Scores run in fp8e4 DoubleRow mode: w_qkv's q/k columns are host-reordered so
the projection lands q/k in a "folded" SBUF layout [128, 2, T] where partition
32h+i holds head h's dh pair (i, i+32) at free-slabs o=0/1. A DR matmul then
contracts Ki=32 x Ko=2 = 64 dh per head at 0.5 cycles/row, four heads packed
into the PE array as 32-row tiles. q/k in fp8 costs ~8e-3 end-to-end rel err
(softmax normalization cancels most common-mode error); v/ex stay bf16.

Causal structure: per 512-query chunk, key tiles come in pairs (ktg); the
second diagonal pair is trimmed to the upper half of the query range
(scores/exp/AV all shrink). Masking of the remaining triangle blocks is
[128,128] multiplies on the (otherwise idle) GpSimd/Pool engine, plus zero
memsets for fully-masked blocks.

Softmax denominator rides as a 65th ones-column of V (row 64 of the AV psum).
Normalization: 2 DVE reciprocals -> one packed f32r K=2 broadcast matmul
([2,512] -> [128,512] via 0/1 selector lhsT) -> 2 DVE muls.

This container's walrus rejects >1 sync wait per instruction, so we
post-process the BIR JSON to hoist extra waits into standalone
EventSemaphore instructions (see _split_multi_waits_json).
"""

import json

import numpy as np
import ml_dtypes

import concourse.bass as bass
import concourse.mybir as mybir
from concourse.tile import TileContext
from concourse.bass_utils import run_bass_kernel_spmd

BF16 = ml_dtypes.bfloat16
F8NP = ml_dtypes.float8_e4m3

# Set by test harnesses: trace=True captures NTFF profile; LAST_RESULTS holds
# the BassKernelResults of the most recent kernel() call.
TRACE = False
LAST_RESULTS = None

# Benchmarking knob: emit the whole pipeline REPEAT times in one NEFF so
# slope differencing isolates one pipeline execution from dispatch overhead.
REPEAT = 1

B, T, C = 4, 2048, 1024
H, DH = 16, 64
HL = 8  # heads per core
HDL = HL * DH  # 512 local head channels
QC = 512  # query-chunk width (PSUM bank limit for fp32 matmul out)
NQC = T // QC  # 4
NKT = T // 128  # 16 key tiles
N_CORES = 8

F32 = mybir.dt.float32
F32R = mybir.dt.float32r
BF = mybir.dt.bfloat16
F8 = mybir.dt.float8e4
DR = mybir.MatmulPerfMode.DoubleRow


def _split_multi_waits_json(raw: bytes) -> bytes:
    """Walrus here supports at most ONE sync wait per instruction. Hoist
    extras into standalone single-wait EventSemaphore instructions inserted
    immediately before, on the same engine (sequencers run in order, so
    waiting sequentially == waiting on all). Drains get ALL waits hoisted."""
    mod = json.loads(raw)
    ctr = 0
    for f in mod.get("functions", []):
        for blk in f.get("blocks", []):
            out = []
            changed = False
            for inst in blk.get("instructions", []):
                si = inst.get("sync_info")
                if si:
                    waits = si.get("on_wait") or []
                    keep = 0 if inst.get("opcode") == "Drain" else 1
                    if len(waits) > keep:
                        for w in waits[: len(waits) - keep]:
                            ctr += 1
                            out.append(
                                {
                                    "name": f"hoisted_wait_{ctr}",
                                    "engine": inst["engine"],
                                    "opcode": "EventSemaphore",
                                    "ins": [],
                                    "outs": [],
                                    "sync_info": {"on_wait": [w], "on_update": []},
                                }
                            )
                        si["on_wait"] = waits[len(waits) - keep :]
                        changed = True
                out.append(inst)
            if changed:
                blk["instructions"] = out
    return json.dumps(mod).encode()


def _build_nc(with_qk_bias: bool, with_v_bias: bool) -> bass.Bass:
    nc = bass.Bass("TRN2", target_bir_lowering=False)

    xt_d = nc.dram_tensor("xt", [C, T], BF, kind="ExternalInput")
    wqk_d = nc.dram_tensor("wqk", [C, 1024], BF, kind="ExternalInput")
    bqk_d = nc.dram_tensor("bqk", [128, 8], F32, kind="ExternalInput")
    wv_d = nc.dram_tensor("wv", [C, HDL], BF, kind="ExternalInput")
    bv_d = nc.dram_tensor("bv", [1, HDL], BF, kind="ExternalInput")
    wout_d = nc.dram_tensor("wout", [HDL, C], BF, kind="ExternalInput")
    mask_d = nc.dram_tensor("mask", [128, 128], BF, kind="ExternalInput")
    out_d = nc.dram_tensor("out_t", [C, T], BF, kind="ExternalOutput")

    exp_f = mybir.ActivationFunctionType.Exp

    with TileContext(nc) as tc:
        with (
            tc.tile_pool(name="consts", bufs=1) as consts,
            tc.tile_pool(name="ps_s", bufs=2, space="PSUM") as ps_s,
            tc.tile_pool(name="ps_y", bufs=2, space="PSUM") as ps_y,
            tc.tile_pool(name="ps_o", bufs=2, space="PSUM") as ps_o,
            tc.tile_pool(name="work", bufs=4) as work,
            tc.tile_pool(name="small", bufs=2) as small,
            tc.tile_pool(name="ostage", bufs=3) as ostage,
        ):
            xt_sb = [consts.tile([128, T], BF, name=f"xt_sb{i}") for i in range(8)]
            wqk_sb = [consts.tile([128, 1024], BF, name=f"wqk_sb{i}") for i in range(8)]
            wv_sb = [consts.tile([128, HDL], BF, name=f"wv_sb{i}") for i in range(8)]
            wout_sb = [consts.tile([128, C], BF, name=f"wout_sb{i}") for i in range(4)]
            bqk_sb = consts.tile([128, 8], F32, name="bqk_sb")
            bv_sb = consts.tile([1, HDL], BF, name="bv_sb")
            mask_sb = consts.tile([128, 128], BF, name="mask_sb")
            ones128 = consts.tile([1, 128], BF, name="ones128")
            self_f = consts.tile([1, 128], F32, name="self_f")
            sel_h = [consts.tile([1, 128], F32R, name=f"sel_h{i}") for i in range(2)]
            # folded q/k quads: partition 32h+i = head (4*quad+h), dh i+32o
            qt_q = [consts.tile([128, 2, T], F8, name=f"qt_quad{q}") for q in range(2)]
            kt_q = [consts.tile([128, 2, T], F8, name=f"kt_quad{q}") for q in range(2)]
            vs = [consts.tile([128, HL, 65], BF, name=f"vs{t}") for t in range(NKT)]
            yt_p = [consts.tile([128, T], BF, name=f"yt_pair{p}") for p in range(4)]

            # Load order: first chunk of every xt tile + all wqk so the first
            # projection chains start ASAP; bulk follows; wout (needed last)
            # goes last.
            for i in range(8):
                nc.sync.dma_start(out=wqk_sb[i], in_=wqk_d[128 * i : 128 * (i + 1), :])
                nc.sync.dma_start(
                    out=xt_sb[i][:, 0:QC], in_=xt_d[128 * i : 128 * (i + 1), 0:QC]
                )
            nc.sync.dma_start(out=bqk_sb, in_=bqk_d[:, :])
            for i in range(8):
                nc.sync.dma_start(out=wv_sb[i], in_=wv_d[128 * i : 128 * (i + 1), :])
            nc.sync.dma_start(out=bv_sb, in_=bv_d[:, :])
            nc.sync.dma_start(out=mask_sb, in_=mask_d[:, :])
            for nch in range(1, 4):
                for i in range(8):
                    nc.sync.dma_start(
                        out=xt_sb[i][:, QC * nch : QC * (nch + 1)],
                        in_=xt_d[128 * i : 128 * (i + 1), QC * nch : QC * (nch + 1)],
                    )
            for i in range(4):
                nc.sync.dma_start(out=wout_sb[i], in_=wout_d[128 * i : 128 * (i + 1), :])
            nc.vector.memset(ones128, 1.0)
            for i in range(2):
                nc.vector.memset(self_f[0:1, 0:64], 1.0 if i == 0 else 0.0)
                nc.vector.memset(self_f[0:1, 64:128], 0.0 if i == 0 else 1.0)
                with nc.allow_low_precision(reason="exact 0/1 to f32r"):
                    nc.vector.tensor_copy(out=sel_h[i], in_=self_f)
            for t in range(NKT):
                nc.vector.memset(vs[t][:, :, 64:65], 1.0)

            def qk_chunk(mt, nch):
                # mt 0-3: Q (quad mt//2, dh-half mt%2); mt 4-7: K likewise.
                ps = ps_o.tile([128, QC], F32, tag="proj", name=f"psqk{mt}_{nch}")
                for kt in range(8):
                    nc.tensor.matmul(
                        out=ps,
                        lhsT=wqk_sb[kt][:, 128 * mt : 128 * (mt + 1)],
                        rhs=xt_sb[kt][:, QC * nch : QC * (nch + 1)],
                        start=(kt == 0),
                        stop=(kt == 7),
                    )
                dest = (qt_q if mt < 4 else kt_q)[(mt % 4) // 2]
                dslice = dest[:, mt % 2, QC * nch : QC * (nch + 1)]
                with nc.allow_low_precision(reason="q/k to fp8 for DR scores"):
                    if with_qk_bias:
                        nc.vector.tensor_scalar_add(
                            out=dslice, in0=ps, scalar1=bqk_sb[:, mt : mt + 1]
                        )
                    else:
                        nc.vector.tensor_copy(out=dslice, in_=ps)

            def v_proj(tt):
                ps = ps_o.tile([128, HDL], F32, tag="proj", name=f"psv{tt}")
                for kt in range(8):
                    nc.tensor.matmul(
                        out=ps,
                        lhsT=xt_sb[kt][:, 128 * tt : 128 * (tt + 1)],
                        rhs=wv_sb[kt],
                        start=(kt == 0),
                        stop=(kt == 7 and not with_v_bias),
                    )
                if with_v_bias:
                    nc.tensor.matmul(
                        out=ps, lhsT=ones128, rhs=bv_sb, start=False, stop=True
                    )
                nc.vector.tensor_copy(
                    out=vs[tt][:, :, 0:64],
                    in_=ps.rearrange("p (h d) -> p h d", h=HL),
                )

            def attention(qc, pair):
                # Generator: yields after each k-tile group so filler PE work
                # can be woven between groups (keeps PE fed while ACT exps).
                # pair p: heads (2p, 2p+1); quad q=p//2, blocks a=64*(p%2).
                quad = pair // 2
                blk = 64 * (pair % 2)
                n_kt = 4 * (qc + 1)  # causal: keys up to this q-chunk
                y_ps = [
                    ps_y.tile([65, QC], F32, tag="y", name=f"y{qc}_{pair}_{h}")
                    for h in (0, 1)
                ]
                for ktg in range(n_kt // 2):
                    kts = (2 * ktg, 2 * ktg + 1)
                    # second diagonal pair: queries < 256 can't see these keys
                    lo = 256 if kts[0] == 4 * qc + 2 else 0
                    s_tiles = []
                    for half in (0, 1):
                        base = blk + 32 * half
                        s_ps = ps_s.tile(
                            [128, 2, QC], F32, tag="s",
                            name=f"s{qc}_{pair}_{ktg}_{half}",
                        )
                        s_tiles.append(s_ps)
                        for j, kt in enumerate(kts):
                            nc.tensor.matmul(
                                out=s_ps[:, j, lo:QC],
                                lhsT=kt_q[quad][base : base + 32, :, 128 * kt : 128 * (kt + 1)],
                                rhs=qt_q[quad][base : base + 32, :, QC * qc + lo : QC * (qc + 1)],
                                start=True,
                                stop=True,
                                perf_mode=DR,
                                tile_position=(base, 0),
                            )
                    for half in (0, 1):
                        h = 2 * pair + half
                        s_ps = s_tiles[half]
                        ex = work.tile(
                            [128, 2, QC], BF, tag="ex", bufs=6,
                            name=f"ex{qc}_{pair}_{ktg}_{half}",
                        )
                        nc.scalar.activation(
                            out=ex[:, :, lo:QC], in_=s_ps[:, :, lo:QC],
                            func=exp_f, scale=0.125,
                        )
                        for j, kt in enumerate(kts):
                            r = kt - 4 * qc
                            if 0 <= r <= 3:
                                if 128 * r > lo:
                                    # keys entirely above all queries in range
                                    nc.gpsimd.memset(ex[:, j, lo : 128 * r], 0.0)
                                nc.gpsimd.tensor_mul(
                                    ex[:, j, 128 * r : 128 * (r + 1)],
                                    ex[:, j, 128 * r : 128 * (r + 1)],
                                    mask_sb,
                                )
                        for j, kt in enumerate(kts):
                            nc.tensor.matmul(
                                out=y_ps[half][:, lo:QC],
                                lhsT=vs[kt][:, h, :],
                                rhs=ex[:, j, lo:QC],
                                start=(kt == 0),
                                stop=(kt == n_kt - 1),
                            )
                    yield
                # normalize: reciprocals of both denoms, then one PSUM
                # accumulation group of two selector matmuls broadcasts them
                # to partition halves of br.
                r2 = [
                    small.tile([1, QC], F32R, tag="r", name=f"r{qc}_{pair}_{h}")
                    for h in (0, 1)
                ]
                with nc.allow_low_precision(reason="softmax denom recip"):
                    for half in (0, 1):
                        nc.vector.reciprocal(
                            out=r2[half], in_=y_ps[half][64:65, :]
                        )
                br = ps_o.tile([128, QC], F32, tag="proj", name=f"br{qc}_{pair}")
                for half in (0, 1):
                    nc.tensor.matmul(
                        out=br, lhsT=sel_h[half], rhs=r2[half],
                        start=(half == 0), stop=(half == 1),
                    )
                br_sb = work.tile([128, QC], F32, tag="brsb", bufs=2, name=f"brsb{qc}_{pair}")
                nc.vector.tensor_copy(out=br_sb, in_=br)
                for half in (0, 1):
                    nc.vector.tensor_mul(
                        out=yt_p[pair][64 * half : 64 * (half + 1), QC * qc : QC * (qc + 1)],
                        in0=y_ps[half][0:64, :],
                        in1=br_sb[64 * half : 64 * (half + 1), :],
                    )

            def outproj(mt, nch):
                ps = ps_o.tile([128, QC], F32, tag="proj", name=f"pso{mt}_{nch}")
                for kt in range(4):
                    nc.tensor.matmul(
                        out=ps,
                        lhsT=wout_sb[kt][:, 128 * mt : 128 * (mt + 1)],
                        rhs=yt_p[kt][:, QC * nch : QC * (nch + 1)],
                        start=(kt == 0),
                        stop=(kt == 3),
                    )
                ob = ostage.tile([128, QC], BF, tag="ob", name=f"ob{mt}_{nch}")
                with nc.allow_low_precision(reason="bf16 output"):
                    nc.vector.tensor_copy(out=ob, in_=ps)
                nc.sync.dma_start(
                    out=out_d[128 * mt : 128 * (mt + 1), QC * nch : QC * (nch + 1)],
                    in_=ob,
                )

            def weave(qc, pair, fillers):
                # Drive the attention generator, spreading filler emissions
                # evenly between its k-tile groups.
                g = attention(qc, pair)
                n = 2 * (qc + 1)
                m = len(fillers)
                done = 0
                for i in range(n):
                    next(g)
                    want = ((i + 1) * m) // n
                    while done < want:
                        fillers[done]()
                        done += 1
                for _ in g:  # tail (normalize) emission
                    pass
                while done < m:
                    fillers[done]()
                    done += 1

            def QK(mt, nch):
                return lambda: qk_chunk(mt, nch)

            def V(tt):
                return lambda: v_proj(tt)

            def OP(mt, nch):
                return lambda: outproj(mt, nch)

            # Filler schedule: each attention instance (pair, qc) carries the
            # PE-only work whose results are needed one-or-more instances
            # later, so PE never drains while ACT is the local bottleneck.
            # Quad 0 uses wqk chunks {0,1} (Q) / {4,5} (K); quad 1 {2,3}/{6,7}.
            fills = {
                (0, 0): [QK(0, 1), QK(1, 1), QK(4, 1), QK(5, 1), V(4), V(5), V(6), V(7)],
                (0, 1): [QK(0, 2), QK(1, 2), QK(4, 2), QK(5, 2), V(8), V(9), V(10), V(11)],
                (0, 2): [QK(0, 3), QK(1, 3), QK(4, 3), QK(5, 3), V(12), V(13), V(14), V(15)],
                (0, 3): [QK(2, 0), QK(3, 0)],
                (1, 0): [QK(6, 0), QK(7, 0)],
                (1, 1): [QK(2, 1), QK(3, 1)],
                (1, 2): [QK(6, 1), QK(7, 1)],
                (1, 3): [QK(2, 2), QK(3, 2)],
                (2, 0): [QK(6, 2), QK(7, 2)],
                (2, 1): [QK(2, 3), QK(3, 3), QK(6, 3), QK(7, 3)],
                (2, 2): [],
                (2, 3): [],
                (3, 0): [],
                (3, 1): [OP(mt, 0) for mt in range(8)],
                (3, 2): [OP(mt, 1) for mt in range(8)],
                (3, 3): [OP(mt, 2) for mt in range(8)],
            }

            for _rep in range(REPEAT):
                qk_chunk(0, 0)
                qk_chunk(1, 0)
                qk_chunk(4, 0)
                qk_chunk(5, 0)
                for tt in range(4):
                    v_proj(tt)
                for pair in range(4):
                    for qc in range(NQC):
                        weave(qc, pair, fills[(pair, qc)])
                for mt in range(8):
                    outproj(mt, 3)

    orig = nc.to_json_bytes
    nc.to_json_bytes = lambda: _split_multi_waits_json(orig())
    return nc


def _host_shards(x, w_qkv, b_qkv, w_out):
    """Per-core input dicts. Core c: batch c//2, head-group c%2.

    wqk columns are reordered for the dh-folded fp8 layout: chunk 2q+u of the
    Q half holds heads (4q..4q+3) dh[32u:32u+32] (u = dh-half), so that the
    projection's psum partitions 32h+i map directly onto the folded SBUF
    tile's partition 32h+i, free-slab u."""
    mask = np.ascontiguousarray(
        (np.arange(128)[:, None] <= np.arange(128)[None, :]).astype(BF16)
    )

    def fold_cols(w):  # w: [C, 512] (8 heads x 64 dh) -> column-reordered
        wf = w.reshape(C, 8, 2, 32)  # [C, head, dh-half, dh-lo]
        out = np.empty_like(w).reshape(C, 4, 128)
        for q in range(2):
            for u in range(2):
                # chunk 2q+u: heads 4q..4q+3, dh-half u
                out[:, 2 * q + u] = wf[:, 4 * q : 4 * q + 4, u].reshape(C, 128)
        return out.reshape(C, 512)

    def fold_bias(b):  # b: [512] -> [4, 128] (chunk, partition) matching psqk
        bf = b.reshape(8, 2, 32)
        cols = np.empty((4, 128), np.float32)
        for q in range(2):
            for u in range(2):
                cols[2 * q + u] = bf[4 * q : 4 * q + 4, u].reshape(128)
        return cols

    in_maps = []
    for c in range(N_CORES):
        b, g = divmod(c, 2)
        o = 512 * g
        w_q = fold_cols(w_qkv[:, o : o + 512])
        w_k = fold_cols(w_qkv[:, 1024 + o : 1024 + o + 512])
        w_v = w_qkv[:, 2048 + o : 2048 + o + 512]
        bq = fold_bias(b_qkv[o : o + 512])
        bk = fold_bias(b_qkv[1024 + o : 1024 + o + 512])
        bqk = np.concatenate([bq, bk], axis=0).T  # [128, 8]
        in_maps.append(
            {
                "xt": np.ascontiguousarray(x[b].T.astype(BF16)),
                "wqk": np.ascontiguousarray(
                    np.concatenate([w_q, w_k], axis=1).astype(BF16)
                ),
                "bqk": np.ascontiguousarray(bqk.astype(np.float32)),
                "wv": np.ascontiguousarray(w_v.astype(BF16)),
                "bv": np.ascontiguousarray(
                    b_qkv[2048 + o : 2048 + o + 512].reshape(1, 512).astype(BF16)
                ),
                "wout": np.ascontiguousarray(
                    w_out[512 * g : 512 * (g + 1), :].astype(BF16)
                ),
                "mask": mask,
            }
        )
    return in_maps


def kernel(x, w_qkv, b_qkv, w_out, b_out):
    global LAST_RESULTS
    x = np.asarray(x, dtype=np.float32)
    w_qkv = np.asarray(w_qkv, dtype=np.float32)
    b_qkv = np.asarray(b_qkv, dtype=np.float32)
    w_out = np.asarray(w_out, dtype=np.float32)
    b_out = np.asarray(b_out, dtype=np.float32)

    nc = _build_nc(
        with_qk_bias=bool(np.any(b_qkv[:2048] != 0.0)),
        with_v_bias=bool(np.any(b_qkv[2048:] != 0.0)),
    )
    in_maps = _host_shards(x, w_qkv, b_qkv, w_out)
    res = run_bass_kernel_spmd(
        nc, in_maps, core_ids=list(range(N_CORES)), trace=TRACE
    )
    LAST_RESULTS = res

    out = np.empty((B, T, C), np.float32)
    for b in range(B):
        p = res.results[2 * b]["out_t"].astype(np.float32) + res.results[
            2 * b + 1
        ]["out_t"].astype(np.float32)
        out[b] = p.T + b_out[None, :]
    return out
